# revision 1
# baseline (speedup 1.0000x reference)
"""Distributed Trainium2 Bass kernel for nn_Attention_25460566131147.

Multi-head attention (B=4, TQ=T=2048, E=2048, H=16, D=128) with gather-based
RoPE and key masking, sharded over 8 NeuronCores: data-parallel over batch
(4 groups) x tensor-parallel over heads (2-way: Wq/Wk/Wv column shards,
Wo row shards, AllReduce of the out-projection partials inside each pair).

Device algorithm (per core, all matmuls bf16 with f32 PSUM accumulation):
  - activations are kept feature-on-partitions (x^T layouts, prepared on host)
  - scores are computed transposed (S^T[k,q] = K_h^T-slice^T @ Q_h^T) so the
    exp'd tile P^T feeds the P@V matmul directly (no on-chip transposes)
  - exp via ScalarE activation with the key-mask folded into the per-partition
    bias and the 1/sqrt(D) scale folded into the activation scale; softmax max-
    subtraction is skipped (scores are O(6), fp32 exp is exact enough)
  - softmax denominator via a ones-column matmul accumulated alongside P@V;
    reciprocal on VectorE once; broadcast back via DMA (engines are 128-lane
    lockstep and cannot cross partitions; DMA can)
  - rotate-half for RoPE via two small SBUF->SBUF DMAs (partition rotation)
"""

import os
import sys

if "JAX_PLATFORMS" in os.environ and os.environ["JAX_PLATFORMS"] == "axon":
    os.environ["JAX_PLATFORMS"] = "axon,cpu"
sys.path.insert(0, "/opt/trn_rl_repo")

import numpy as np
import ml_dtypes

BF16NP = ml_dtypes.bfloat16

B, TQ, T, E, H, D = 4, 2048, 2048, 2048, 16, 128
BLOCK, THETA = 4096, 10000.0
N_CORES = 8
P = 128

FULL_CFG = dict(TQ=TQ, T=T, E=E, HL=8, D=D, NCORES=N_CORES)


def _cs(total, w):
    """Column splits: list of (start, width)."""
    return [(i, min(w, total - i)) for i in range(0, total, w)]


def build_nc(cfg=None):
    """Build and return the (uncompiled) Bacc graph for one SPMD core."""
    import concourse.mybir as mybir
    import concourse.tile as tile
    from concourse import bacc
    from contextlib import ExitStack

    c = dict(FULL_CFG)
    if cfg:
        c.update(cfg)
    cTQ, cT, cE, HL, cD, NCORES = (
        c["TQ"], c["T"], c["E"], c["HL"], c["D"], c["NCORES"],
    )
    assert cD == P
    F = HL * cD              # local feature width (heads shard)
    EC = cE // P             # contraction chunks for projections
    TC = cT // P             # key-position chunks
    NQ = min(512, cTQ)       # q-tile width (PSUM bank limit)
    BF = mybir.dt.bfloat16
    F32 = mybir.dt.float32
    SCALE = 1.0 / float(np.sqrt(cD))
    groups = [[2 * i, 2 * i + 1] for i in range(NCORES // 2)]

    nc = bacc.Bacc("TRN2", target_bir_lowering=False, debug=False,
                   num_devices=NCORES)

    xt_d = nc.declare_dram_parameter("xt", [cE, cTQ], BF, isOutput=False)
    xat_d = nc.declare_dram_parameter("xat", [cE, cT], BF, isOutput=False)
    wq_d = nc.declare_dram_parameter("wq", [cE, F], BF, isOutput=False)
    wk_d = nc.declare_dram_parameter("wk", [cE, F], BF, isOutput=False)
    wv_d = nc.declare_dram_parameter("wv", [cE, F], BF, isOutput=False)
    wo_d = nc.declare_dram_parameter("wo", [F, cE], BF, isOutput=False)
    cosq_d = nc.declare_dram_parameter("cosq", [P, cTQ], BF, isOutput=False)
    sinq_d = nc.declare_dram_parameter("sinq", [P, cTQ], BF, isOutput=False)
    cosk_d = nc.declare_dram_parameter("cosk", [P, cT], BF, isOutput=False)
    sink_d = nc.declare_dram_parameter("sink", [P, cT], BF, isOutput=False)
    mb_d = nc.declare_dram_parameter("mbias", [P, TC], F32, isOutput=False)
    NT = cE // P                      # out-projection row tiles
    NCH = 8 if NT % 8 == 0 else (2 if NT % 2 == 0 else 1)
    CR = (NT // NCH) * P              # chunk rows (collective granule)
    out_d = nc.declare_dram_parameter("out", [cE // 2, cTQ], BF, isOutput=True)

    obounce = [nc.dram_tensor(f"obounce{ch}", [CR, cTQ], BF)
               for ch in range(NCH)]
    orsc = [nc.dram_tensor(f"orsc{ch}", [CR // 2, cTQ], BF)
            for ch in range(NCH)]

    with tile.TileContext(nc) as tc, ExitStack() as ex:
        # right side: persistent accumulating tiles; left side: phase-scoped
        consts = ex.enter_context(tc.tile_pool(name="consts", bufs=1, side="right"))
        ones_bf = consts.tile([P, 1], BF, tag="ones_bf", name="ones_bf")
        nc.vector.memset(ones_bf[:], 1.0)
        mb_sb = consts.tile([P, TC], F32, tag="mbias", name="mbias")
        nc.sync.dma_start(mb_sb[:], mb_d[:])
        # packed denominators: head m lives at partition base (m%4)*32
        # (engine ops need 32-aligned start partitions), column (m//4)*128
        den_sb = consts.tile([P, 2 * P], F32, tag="den", name="den")
        ones_fr = consts.tile([1, P], F32, tag="ones_fr", name="ones_fr")
        nc.vector.memset(ones_fr[:], 1.0)

        SEG = min(512, cT)   # projection/rope column-segment width

        def proj_rope(m, w_sb, src_tiles, src_c0, out_c0, width, cos_sb,
                      sin_sb, out_t, tg, rawp, tmpp, psproj):
            """Project head m (cols [src_c0, src_c0+width) of src) and apply
            RoPE, writing cols [out_c0, out_c0+width) of out_t."""
            raw = rawp.tile([P, width], BF, tag=f"raw{tg}", name=f"raw{tg}")
            swp = rawp.tile([P, width], BF, tag=f"swp{tg}", name=f"swp{tg}")
            ps = psproj.tile([P, SEG], F32, tag="projps", name="projps")
            for e in range(EC):
                for ns, nw in _cs(width, 512):
                    nc.tensor.matmul(
                        ps[:, ns:ns + nw],
                        w_sb[e][:, m * P:(m + 1) * P],
                        src_tiles[e][:, src_c0 + ns:src_c0 + ns + nw],
                        start=(e == 0), stop=(e == EC - 1),
                    )
            nc.scalar.copy(raw[:], ps[:, 0:width])
            # partition rotate-half via SBUF->SBUF DMA (cross-partition)
            half = P // 2
            nc.sync.dma_start(swp[0:half, :], raw[half:P, :])
            nc.sync.dma_start(swp[half:P, :], raw[0:half, :])
            t1 = tmpp.tile([P, width], BF, tag="rope_t1", name="rope_t1")
            t2 = tmpp.tile([P, width], BF, tag="rope_t2", name="rope_t2")
            nc.vector.tensor_mul(t1[:], raw[:], cos_sb[:, out_c0:out_c0 + width])
            nc.vector.tensor_mul(t2[:], swp[:], sin_sb[:, out_c0:out_c0 + width])
            nc.vector.tensor_add(out_t[:, out_c0:out_c0 + width], t1[:], t2[:])

        vp = ex.enter_context(tc.tile_pool(name="v", bufs=1, side="right"))
        ktp = ex.enter_context(tc.tile_pool(name="kt", bufs=1, side="right"))
        qtp = ex.enter_context(tc.tile_pool(name="qt", bufs=1, side="right"))

        es_proj = ExitStack()   # projection psum: [V .. Q]
        psproj = es_proj.enter_context(
            tc.tile_pool(name="psproj", bufs=2, space="PSUM"))
        # prefetch pools opened early so their DMA loads are not WAR-blocked
        # behind the previous phase's tiles
        es_xt = ExitStack()     # x^T quarters: [V .. Q]
        xtp = es_xt.enter_context(tc.tile_pool(name="xt", bufs=1))
        es_xak = ExitStack()    # xall^T quarters for K: [V .. K]
        xakp = es_xak.enter_context(tc.tile_pool(name="xak", bufs=1))

        # ============ phase V: V = xall @ Wv, [t-part, n-free] ===========
        # xall^T is streamed in 1024-column halves (and re-streamed for K)
        # to bound SBUF.
        assert F <= 1024
        v_sb = [vp.tile([P, F], BF, tag=f"v{t}", name=f"v{t}")
                for t in range(TC)]
        with tc.tile_pool(name="wv", bufs=1) as wvp, \
                tc.tile_pool(name="xav", bufs=1) as xavp:
            wv_sb = []
            for e in range(EC):
                t_ = wvp.tile([P, F], BF, tag=f"wv{e}", name=f"wv{e}")
                nc.sync.dma_start(t_[:], wv_d[e * P:(e + 1) * P, :])
                wv_sb.append(t_)
            for h0, hw in _cs(cT, SEG):
                xa_sb = []
                for e in range(EC):
                    t_ = xavp.tile([P, SEG], BF, tag=f"xav{e}", name=f"xav{e}")
                    nc.sync.dma_start(
                        t_[:, 0:hw], xat_d[e * P:(e + 1) * P, h0:h0 + hw])
                    xa_sb.append(t_)
                for tl in range(hw // P):
                    t = (h0 // P) + tl
                    ps = psproj.tile([P, F], F32, tag="projpsv", name="projpsv")
                    for e in range(EC):
                        for ns, nw in _cs(F, 512):
                            nc.tensor.matmul(
                                ps[:, ns:ns + nw],
                                xa_sb[e][:, tl * P:(tl + 1) * P],
                                wv_sb[e][:, ns:ns + nw],
                                start=(e == 0), stop=(e == EC - 1),
                            )
                    nc.scalar.copy(v_sb[t][:], ps[:, 0:F])

        # ============ phase K: K-proj + RoPE =============================
        kt_sb = [ktp.tile([P, cT], BF, tag=f"kt{m}", name=f"kt{m}")
                 for m in range(HL)]
        with tc.tile_pool(name="tabk", bufs=1) as tabk, \
                tc.tile_pool(name="wk", bufs=1) as wkp, \
                tc.tile_pool(name="rawk", bufs=1) as rawkp, \
                tc.tile_pool(name="tmpk", bufs=2) as tmpkp:
            cosk_sb = tabk.tile([P, cT], BF, tag="cosk", name="cosk")
            sink_sb = tabk.tile([P, cT], BF, tag="sink", name="sink")
            nc.sync.dma_start(cosk_sb[:], cosk_d[:])
            nc.sync.dma_start(sink_sb[:], sink_d[:])
            wk_sb = []
            for e in range(EC):
                t_ = wkp.tile([P, F], BF, tag=f"wk{e}", name=f"wk{e}")
                nc.sync.dma_start(t_[:], wk_d[e * P:(e + 1) * P, :])
                wk_sb.append(t_)
            for h0, hw in _cs(cT, SEG):
                xa_sb = []
                for e in range(EC):
                    t_ = xakp.tile([P, SEG], BF, tag=f"xak{e}", name=f"xak{e}")
                    nc.sync.dma_start(
                        t_[:, 0:hw], xat_d[e * P:(e + 1) * P, h0:h0 + hw])
                    xa_sb.append(t_)
                for m in range(HL):
                    proj_rope(m, wk_sb, xa_sb, 0, h0, hw, cosk_sb,
                              sink_sb, kt_sb[m], "k", rawkp, tmpkp, psproj)
        es_xak.close()

        # ============ phase Q: Q-proj + RoPE (x^T in halves) =============
        qt_sb = []
        for m in range(HL):
            qt_sb.append(qtp.tile([P, cTQ], BF, tag=f"qt{m}", name=f"qt{m}"))
        with tc.tile_pool(name="wq", bufs=1) as wqp, \
                tc.tile_pool(name="tabq", bufs=1) as tabq, \
                tc.tile_pool(name="rawq", bufs=1) as rawqp, \
                tc.tile_pool(name="tmpq", bufs=2) as tmpqp:
            cosq_sb = tabq.tile([P, cTQ], BF, tag="cosq", name="cosq")
            sinq_sb = tabq.tile([P, cTQ], BF, tag="sinq", name="sinq")
            nc.sync.dma_start(cosq_sb[:], cosq_d[:])
            nc.sync.dma_start(sinq_sb[:], sinq_d[:])
            wq_sb = []
            for e in range(EC):
                t_ = wqp.tile([P, F], BF, tag=f"wq{e}", name=f"wq{e}")
                nc.sync.dma_start(t_[:], wq_d[e * P:(e + 1) * P, :])
                wq_sb.append(t_)
            TQH = min(512, cTQ)
            for th, (h0, hw) in enumerate(_cs(cTQ, TQH)):
                xt_sb = []
                for e in range(EC):
                    t_ = xtp.tile([P, TQH], BF, tag=f"xt{e}", name=f"xt{e}")
                    nc.sync.dma_start(
                        t_[:], xt_d[e * P:(e + 1) * P, h0:h0 + hw])
                    xt_sb.append(t_)
                for m in range(HL):
                    proj_rope(m, wq_sb, xt_sb, 0, h0, hw, cosq_sb, sinq_sb,
                              qt_sb[m], "q", rawqp, tmpqp, psproj)
        es_xt.close()
        es_proj.close()

        # ================= phase C: attention ============================
        es_yt = ExitStack()     # yt tiles: [C .. end of D] (normalized in place)
        ytp = es_yt.enter_context(tc.tile_pool(name="yt", bufs=1))
        yt_sb = []
        for m in range(HL):
            yt_sb.append(ytp.tile([P, cTQ], BF, tag=f"yt{m}", name=f"yt{m}"))

        FR = mybir.dt.float32r
        RPM = cTQ // P                # packed den rows per head
        with tc.tile_pool(name="pt", bufs=TC + 2) as ptp, \
                tc.tile_pool(name="pt2", bufs=TC // 2 + 2) as pt2p, \
                tc.tile_pool(name="dst", bufs=2) as dstp, \
                tc.tile_pool(name="dner", bufs=2) as dnerp, \
                tc.tile_pool(name="pss", bufs=3, space="PSUM") as pss, \
                tc.tile_pool(name="psy", bufs=2, space="PSUM") as psy, \
                tc.tile_pool(name="psd", bufs=2, space="PSUM") as psd, \
                tc.tile_pool(name="psb", bufs=1, space="PSUM") as psb:
            for m in range(HL):
                for qs, qw in _cs(cTQ, NQ):
                    yps = psy.tile([P, NQ], F32, tag="yps", name="yps")
                    dps = psd.tile([1, NQ], F32, tag="dps", name="dps")
                    pts = []
                    for kc in range(TC):
                        sps = pss.tile([P, NQ], F32, tag="sps", name="sps")
                        nc.tensor.matmul(
                            sps[:, 0:qw],
                            kt_sb[m][:, kc * P:(kc + 1) * P],
                            qt_sb[m][:, qs:qs + qw],
                            start=True, stop=True,
                        )
                        pt = ptp.tile([P, NQ], BF, tag="pt", name="pt")
                        pts.append(pt)
                        nc.scalar.activation(
                            pt[:, 0:qw], sps[:, 0:qw],
                            mybir.ActivationFunctionType.Exp,
                            bias=mb_sb[:, kc:kc + 1], scale=SCALE,
                        )
                        nc.tensor.matmul(
                            yps[:, 0:qw],
                            v_sb[kc][:, m * P:(m + 1) * P],
                            pt[:, 0:qw],
                            start=(kc == 0), stop=(kc == TC - 1),
                        )
                    # denominator: pair-sum the exp'd chunks on the idle
                    # GpSimd/Vector engines, then TC/2 ones-matmuls
                    assert TC % 2 == 0
                    pt2s = []
                    for i in range(TC // 2):
                        pt2 = pt2p.tile([P, NQ], BF, tag="pt2", name="pt2")
                        pt2s.append(pt2)
                        eng = nc.gpsimd if (i % 2 == 0) else nc.vector
                        eng.tensor_add(pt2[:, 0:qw], pts[2 * i][:, 0:qw],
                                       pts[2 * i + 1][:, 0:qw])
                    for i in range(TC // 2):
                        nc.tensor.matmul(
                            dps[0:1, 0:qw],
                            ones_bf[:, 0:1],
                            pt2s[i][:, 0:qw],
                            start=(i == 0), stop=(i == TC // 2 - 1),
                        )
                    nc.vector.tensor_copy(yt_sb[m][:, qs:qs + qw], yps[:, 0:qw])
                    dst = dstp.tile([1, NQ], F32, tag="dst", name="dst")
                    nc.vector.tensor_copy(dst[0:1, 0:qw], dps[0:1, 0:qw])
                    # scatter the denominator row into the packed layout
                    # (DMA can cross partitions)
                    bp = (m % 4) * 32 + qs // P
                    c0 = (m // 4) * P
                    nc.sync.dma_start(
                        den_sb[bp:bp + qw // P, c0:c0 + P], dst[0:1, 0:qw])
                # ---- head m normalization (overlaps head m+1 attention) ----
                bp = (m % 4) * 32
                c0 = (m // 4) * P
                nc.vector.reciprocal(den_sb[bp:bp + RPM, c0:c0 + P],
                                     den_sb[bp:bp + RPM, c0:c0 + P])
                dner = dnerp.tile([1, cTQ], F32, tag="dner", name="dner")
                nc.sync.dma_start(dner[0:1, :],
                                  den_sb[bp:bp + RPM, c0:c0 + P])
                for qs, qw in _cs(cTQ, NQ):
                    dbc = psb.tile([P, NQ], F32, tag="dbc", name="dbc")
                    nc.tensor.matmul(
                        dbc[:, 0:qw],
                        ones_fr[0:1, :].bitcast(FR),
                        dner[0:1, qs:qs + qw].bitcast(FR),
                        start=True, stop=True,
                    )
                    nc.vector.tensor_mul(
                        yt_sb[m][:, qs:qs + qw],
                        yt_sb[m][:, qs:qs + qw],
                        dbc[:, 0:qw],
                    )

        es_wo = ExitStack()     # wo tiles: [D]
        wop = es_wo.enter_context(tc.tile_pool(name="wo", bufs=1))
        wo_sb = []
        for f in range(HL):
            t_ = wop.tile([P, cE], BF, tag=f"wo{f}", name=f"wo{f}")
            nc.sync.dma_start(t_[:], wo_d[f * P:(f + 1) * P, :])
            wo_sb.append(t_)

        # ================= phase D: out-projection =======================
        # emitted in NCH chunks of CR rows of E; each chunk's partials are
        # ReduceScattered within the pair while the next chunk computes
        with tc.tile_pool(name="oev", bufs=4) as oevp, \
                tc.tile_pool(name="pso", bufs=2, space="PSUM") as pso:
            for ch in range(NCH):
                for nl in range(CR // P):
                    n = ch * (CR // P) + nl
                    for ms, mw in _cs(cTQ, 512):
                        ops = pso.tile([P, 512], F32, tag="ops", name="ops")
                        for f in range(HL):
                            nc.tensor.matmul(
                                ops[:, 0:mw],
                                wo_sb[f][:, n * P:(n + 1) * P],
                                yt_sb[f][:, ms:ms + mw],
                                start=(f == 0), stop=(f == HL - 1),
                            )
                        oev = oevp.tile([P, 512], BF, tag="oev", name="oev")
                        nc.scalar.copy(oev[:, 0:mw], ops[:, 0:mw])
                        nc.sync.dma_start(
                            obounce[ch][nl * P:(nl + 1) * P, ms:ms + mw],
                            oev[:, 0:mw])
                nc.gpsimd.collective_compute(
                    "ReduceScatter",
                    mybir.AluOpType.add,
                    replica_groups=groups,
                    ins=[obounce[ch][:]],
                    outs=[orsc[ch][:]],
                )
                nc.sync.dma_start(
                    out_d[ch * (CR // 2):(ch + 1) * (CR // 2), :],
                    orsc[ch][:])
        es_wo.close()
        es_yt.close()

    return nc




# ---------------------------------------------------------------------------
# host side
# ---------------------------------------------------------------------------

def _rope_tables():
    inv_freq = 1.0 / (THETA ** (np.arange(0, D, 2, dtype=np.float32) / D))
    t = np.arange(BLOCK, dtype=np.float32)
    freqs = np.einsum("i,j->ij", t, inv_freq).astype(np.float32)
    emb = np.concatenate([freqs, freqs], axis=-1)
    return np.cos(emb).astype(np.float32), np.sin(emb).astype(np.float32)


_NC_CACHE = {}


def _get_compiled():
    if "nc" not in _NC_CACHE:
        nc = build_nc()
        nc.compile()
        _NC_CACHE["nc"] = nc
    return _NC_CACHE["nc"]


def _bf(a):
    return np.ascontiguousarray(a).astype(BF16NP)


def prepare_in_maps(x, xall, posx, posxall, mask, Wq, Wk, Wv, Wo):
    x = np.asarray(x, dtype=np.float32)
    xall = np.asarray(xall, dtype=np.float32)
    posx = np.asarray(posx)
    posxall = np.asarray(posxall)
    mask = np.asarray(mask)
    Wq = np.asarray(Wq, dtype=np.float32)
    Wk = np.asarray(Wk, dtype=np.float32)
    Wv = np.asarray(Wv, dtype=np.float32)
    Wo = np.asarray(Wo, dtype=np.float32)

    cos_t, sin_t = _rope_tables()
    sign = np.ones((1, D), np.float32)
    sign[0, : D // 2] = -1.0

    F = (H * D) // 2  # 1024: per-core head-shard width

    in_maps = []
    for c in range(N_CORES):
        b, hg = c // 2, c % 2
        sl = slice(hg * F, (hg + 1) * F)
        cosq = _bf(cos_t[posx[b]].T)                    # [128, TQ]
        sinq = _bf((sin_t[posx[b]] * sign).T)
        cosk = _bf(cos_t[posxall[b]].T)
        sink = _bf((sin_t[posxall[b]] * sign).T)
        mb = np.where(mask[b], np.float32(-60.0), np.float32(0.0))
        mb = np.ascontiguousarray(mb.reshape(T // P, P).T)  # [128, TC]
        in_maps.append({
            "xt": _bf(x[b].T),
            "xat": _bf(xall[b].T),
            "wq": _bf(Wq[:, sl]),
            "wk": _bf(Wk[:, sl]),
            "wv": _bf(Wv[:, sl]),
            "wo": _bf(Wo[sl, :]),
            "cosq": cosq, "sinq": sinq, "cosk": cosk, "sink": sink,
            "mbias": mb.astype(np.float32),
        })
    return in_maps


def assemble_out(results):
    # ReduceScatter within each pair: chunk ch covers E rows
    # [ch*CR, (ch+1)*CR); rank hg holds the half [ch*CR + hg*CR/2, ...).
    NT = E // P
    NCH = 8 if NT % 8 == 0 else (2 if NT % 2 == 0 else 1)
    CR = (NT // NCH) * P
    out = np.empty((B, TQ, E), np.float32)
    outT = np.empty((E, TQ), np.float32)
    for b in range(B):
        for hg in range(2):
            half = results[2 * b + hg]["out"].astype(np.float32)
            for ch in range(NCH):
                outT[ch * CR + hg * (CR // 2):ch * CR + (hg + 1) * (CR // 2)] = \
                    half[ch * (CR // 2):(ch + 1) * (CR // 2)]
        out[b] = outT.T
    return out


def kernel(x, xall, posx, posxall, mask, Wq, Wk, Wv, Wo):
    from concourse.bass_utils import run_bass_kernel_spmd

    in_maps = prepare_in_maps(x, xall, posx, posxall, mask, Wq, Wk, Wv, Wo)
    nc = _get_compiled()
    res = run_bass_kernel_spmd(nc, in_maps, list(range(N_CORES)), trace=False)
    return assemble_out(res.results)



# revision 2
# speedup vs baseline: 1.2601x; 1.2601x over previous
"""Distributed Trainium2 Bass kernel for nn_Attention_25460566131147.

Multi-head attention (B=4, TQ=T=2048, E=2048, H=16, D=128) with gather-based
RoPE and key masking, sharded over 8 NeuronCores: data-parallel over batch
(4 groups) x tensor-parallel over heads (2-way: Wq/Wk/Wv column shards).

Key optimizations over the straightforward TP scheme:
  - keys are SORTED BY MASK on the host (softmax is permutation-invariant
    over keys): fully-masked key chunks are dropped entirely (~12% of T),
    and all but the last NB chunks carry a uniform zero mask bias
  - uniform-bias chunks let the exp activation read TWO PSUM banks in one
    instruction ([128,1024]), amortizing the ScalarE per-op overhead --
    ScalarE exp is the pacing engine of the attention phase
  - instead of a trailing ReduceScatter of out-projection partials, each
    head's attention output yt is AllGathered within the core pair DURING
    the attention phase (hidden), and each core then runs the FULL
    contraction (all 16 heads) for its half of the output features -- the
    out-projection phase has no collective left to stall on
  - V-proj and K-proj share one streaming pass over xall^T; projection
    input pools are double-buffered so segment boundaries don't stall

Device algorithm details (all matmuls bf16 with f32 PSUM accumulation):
  - activations kept feature-on-partitions (x^T layouts, prepared on host)
  - scores computed transposed (S^T[k,q] = K-chunk^T @ Q^T) so the exp'd
    tile P^T feeds the P@V matmul directly
  - softmax max-subtraction skipped (scores are O(3), fp32 exp is exact
    enough); 1/sqrt(D) folded into the activation scale
  - denominator via pair/quad pre-sums (Vector+GpSimd) + ones-column
    matmuls; reciprocal on VectorE; broadcast back via fp32r matmul
"""

import os
import sys

if "JAX_PLATFORMS" in os.environ and os.environ["JAX_PLATFORMS"] == "axon":
    os.environ["JAX_PLATFORMS"] = "axon,cpu"
sys.path.insert(0, "/opt/trn_rl_repo")

import numpy as np
import ml_dtypes

BF16NP = ml_dtypes.bfloat16

B, TQ, T, E, H, D = 4, 2048, 2048, 2048, 16, 128
BLOCK, THETA = 4096, 10000.0
N_CORES = 8
P = 128

FULL_CFG = dict(TQ=TQ, E=E, HL=8, D=D, NCORES=N_CORES, TKC=14, NB=1)


def _cs(total, w):
    """Column splits: list of (start, width)."""
    return [(i, min(w, total - i)) for i in range(0, total, w)]


def build_nc(cfg=None):
    """Build and return the (uncompiled) Bacc graph for one SPMD core."""
    import concourse.mybir as mybir
    import concourse.tile as tile
    from concourse import bacc
    from contextlib import ExitStack

    c = dict(FULL_CFG)
    if cfg:
        c.update(cfg)
    cTQ, cE, HL, cD, NCORES, TKC, NB = (
        c["TQ"], c["E"], c["HL"], c["D"], c["NCORES"], c["TKC"], c["NB"],
    )
    assert cD == P
    F = HL * cD              # local feature width (heads shard)
    EC = cE // P             # contraction chunks for projections
    TKP = TKC * P            # padded sorted key count
    NQ = min(512, cTQ)       # q-tile width (PSUM bank limit)
    BF = mybir.dt.bfloat16
    F32 = mybir.dt.float32
    SCALE = 1.0 / float(np.sqrt(cD))
    groups = [[2 * i, 2 * i + 1] for i in range(NCORES // 2)]
    NU = TKC - NB            # chunks with uniform zero bias

    nc = bacc.Bacc("TRN2", target_bir_lowering=False, debug=False,
                   num_devices=NCORES)

    xt_d = nc.declare_dram_parameter("xt", [cE, cTQ], BF, isOutput=False)
    xat_d = nc.declare_dram_parameter("xat", [cE, TKP], BF, isOutput=False)
    wq_d = nc.declare_dram_parameter("wq", [cE, F], BF, isOutput=False)
    wk_d = nc.declare_dram_parameter("wk", [cE, F], BF, isOutput=False)
    wv_d = nc.declare_dram_parameter("wv", [cE, F], BF, isOutput=False)
    # wo rows in (head, half) interleaved order, cols = this core's E-half
    wo_d = nc.declare_dram_parameter("wo", [2 * F, cE // 2], BF,
                                     isOutput=False)
    cosq_d = nc.declare_dram_parameter("cosq", [P, cTQ], BF, isOutput=False)
    sinq_d = nc.declare_dram_parameter("sinq", [P, cTQ], BF, isOutput=False)
    cosk_d = nc.declare_dram_parameter("cosk", [P, TKP], BF, isOutput=False)
    sink_d = nc.declare_dram_parameter("sink", [P, TKP], BF, isOutput=False)
    mb_d = nc.declare_dram_parameter("mbias", [P, NB], F32, isOutput=False)
    out_d = nc.declare_dram_parameter("out", [cE // 2, cTQ], BF,
                                      isOutput=True)

    ytd = [nc.dram_tensor(f"ytd{m}", [P, cTQ], BF) for m in range(HL)]
    ytg = [nc.dram_tensor(f"ytg{m}", [2 * P, cTQ], BF) for m in range(HL)]

    with tile.TileContext(nc) as tc, ExitStack() as ex:
        # right side: persistent accumulating tiles; left side: phase-scoped
        consts = ex.enter_context(tc.tile_pool(name="consts", bufs=1,
                                               side="right"))
        ones_bf = consts.tile([P, 1], BF, tag="ones_bf", name="ones_bf")
        nc.vector.memset(ones_bf[:], 1.0)
        mb_sb = consts.tile([P, NB], F32, tag="mbias", name="mbias")
        nc.sync.dma_start(mb_sb[:], mb_d[:])
        # packed denominators: head m lives at partition base (m%4)*32
        # (engine ops need 32-aligned start partitions), column (m//4)*128
        den_sb = consts.tile([P, 2 * P], F32, tag="den", name="den")
        ones_fr = consts.tile([1, P], F32, tag="ones_fr", name="ones_fr")
        nc.vector.memset(ones_fr[:], 1.0)

        vp = ex.enter_context(tc.tile_pool(name="v", bufs=1, side="right"))
        ktp = ex.enter_context(tc.tile_pool(name="kt", bufs=1, side="right"))
        qtp = ex.enter_context(tc.tile_pool(name="qt", bufs=1, side="right"))

        SEG = min(512, TKP)

        # ====== phase VK: V = xall@Wv and K-proj+RoPE in one xat pass ======
        assert F <= 1024
        v_sb = [vp.tile([P, F], BF, tag=f"v{t}", name=f"v{t}")
                for t in range(TKC)]
        kt_sb = [ktp.tile([P, TKP], BF, tag=f"kt{m}", name=f"kt{m}")
                 for m in range(HL)]
        with tc.tile_pool(name="xak", bufs=2) as xakp, \
                tc.tile_pool(name="wv", bufs=1) as wvp, \
                tc.tile_pool(name="wk", bufs=1) as wkp, \
                tc.tile_pool(name="tabk", bufs=1) as tabk, \
                tc.tile_pool(name="rawk", bufs=2) as rawkp, \
                tc.tile_pool(name="tmpk", bufs=2) as tmpkp, \
                tc.tile_pool(name="psv", bufs=2, space="PSUM") as psv, \
                tc.tile_pool(name="psk", bufs=2, space="PSUM") as psk:
            cosk_sb = tabk.tile([P, TKP], BF, tag="cosk", name="cosk")
            sink_sb = tabk.tile([P, TKP], BF, tag="sink", name="sink")
            nc.sync.dma_start(cosk_sb[:], cosk_d[:])
            nc.sync.dma_start(sink_sb[:], sink_d[:])
            wv_sb, wk_sb = [], []
            for e in range(EC):
                t_ = wvp.tile([P, F], BF, tag=f"wv{e}", name=f"wv{e}")
                nc.sync.dma_start(t_[:], wv_d[e * P:(e + 1) * P, :])
                wv_sb.append(t_)
                t_ = wkp.tile([P, F], BF, tag=f"wk{e}", name=f"wk{e}")
                nc.sync.dma_start(t_[:], wk_d[e * P:(e + 1) * P, :])
                wk_sb.append(t_)
            for h0, hw in _cs(TKP, SEG):
                xa_sb = []
                for e in range(EC):
                    t_ = xakp.tile([P, SEG], BF, tag=f"xak{e}",
                                   name=f"xak{e}")
                    nc.sync.dma_start(
                        t_[:, 0:hw], xat_d[e * P:(e + 1) * P, h0:h0 + hw])
                    xa_sb.append(t_)
                # V projection for this segment's key chunks
                for tl in range(hw // P):
                    t = (h0 // P) + tl
                    ps = psv.tile([P, F], F32, tag="psv", name="psv")
                    for e in range(EC):
                        for ns, nw in _cs(F, 512):
                            nc.tensor.matmul(
                                ps[:, ns:ns + nw],
                                xa_sb[e][:, tl * P:(tl + 1) * P],
                                wv_sb[e][:, ns:ns + nw],
                                start=(e == 0), stop=(e == EC - 1),
                            )
                    nc.scalar.copy(v_sb[t][:], ps[:, 0:F])
                # K projection + RoPE for this segment
                for m in range(HL):
                    ps = psk.tile([P, SEG], F32, tag="psk", name="psk")
                    for e in range(EC):
                        nc.tensor.matmul(
                            ps[:, 0:hw],
                            wk_sb[e][:, m * P:(m + 1) * P],
                            xa_sb[e][:, 0:hw],
                            start=(e == 0), stop=(e == EC - 1),
                        )
                    raw = rawkp.tile([P, SEG], BF, tag="rawk", name="rawk")
                    swp = rawkp.tile([P, SEG], BF, tag="swpk", name="swpk")
                    nc.scalar.copy(raw[:, 0:hw], ps[:, 0:hw])
                    half = P // 2
                    nc.sync.dma_start(swp[0:half, 0:hw], raw[half:P, 0:hw])
                    nc.sync.dma_start(swp[half:P, 0:hw], raw[0:half, 0:hw])
                    t1 = tmpkp.tile([P, SEG], BF, tag="t1k", name="t1k")
                    t2 = tmpkp.tile([P, SEG], BF, tag="t2k", name="t2k")
                    nc.vector.tensor_mul(t1[:, 0:hw], raw[:, 0:hw],
                                         cosk_sb[:, h0:h0 + hw])
                    nc.vector.tensor_mul(t2[:, 0:hw], swp[:, 0:hw],
                                         sink_sb[:, h0:h0 + hw])
                    nc.vector.tensor_add(kt_sb[m][:, h0:h0 + hw],
                                         t1[:, 0:hw], t2[:, 0:hw])

        # ============ phase Q: Q-proj + RoPE =============================
        qt_sb = [qtp.tile([P, cTQ], BF, tag=f"qt{m}", name=f"qt{m}")
                 for m in range(HL)]
        with tc.tile_pool(name="xt", bufs=2) as xtp, \
                tc.tile_pool(name="wq", bufs=1) as wqp, \
                tc.tile_pool(name="tabq", bufs=1) as tabq, \
                tc.tile_pool(name="rawq", bufs=2) as rawqp, \
                tc.tile_pool(name="tmpq", bufs=2) as tmpqp, \
                tc.tile_pool(name="psq", bufs=2, space="PSUM") as psq:
            cosq_sb = tabq.tile([P, cTQ], BF, tag="cosq", name="cosq")
            sinq_sb = tabq.tile([P, cTQ], BF, tag="sinq", name="sinq")
            nc.sync.dma_start(cosq_sb[:], cosq_d[:])
            nc.sync.dma_start(sinq_sb[:], sinq_d[:])
            wq_sb = []
            for e in range(EC):
                t_ = wqp.tile([P, F], BF, tag=f"wq{e}", name=f"wq{e}")
                nc.sync.dma_start(t_[:], wq_d[e * P:(e + 1) * P, :])
                wq_sb.append(t_)
            for h0, hw in _cs(cTQ, 512):
                xt_sb = []
                for e in range(EC):
                    t_ = xtp.tile([P, 512], BF, tag=f"xt{e}", name=f"xt{e}")
                    nc.sync.dma_start(
                        t_[:, 0:hw], xt_d[e * P:(e + 1) * P, h0:h0 + hw])
                    xt_sb.append(t_)
                for m in range(HL):
                    ps = psq.tile([P, 512], F32, tag="psq", name="psq")
                    for e in range(EC):
                        nc.tensor.matmul(
                            ps[:, 0:hw],
                            wq_sb[e][:, m * P:(m + 1) * P],
                            xt_sb[e][:, 0:hw],
                            start=(e == 0), stop=(e == EC - 1),
                        )
                    raw = rawqp.tile([P, 512], BF, tag="rawq", name="rawq")
                    swp = rawqp.tile([P, 512], BF, tag="swpq", name="swpq")
                    nc.scalar.copy(raw[:, 0:hw], ps[:, 0:hw])
                    half = P // 2
                    nc.sync.dma_start(swp[0:half, 0:hw], raw[half:P, 0:hw])
                    nc.sync.dma_start(swp[half:P, 0:hw], raw[0:half, 0:hw])
                    t1 = tmpqp.tile([P, 512], BF, tag="t1q", name="t1q")
                    t2 = tmpqp.tile([P, 512], BF, tag="t2q", name="t2q")
                    nc.vector.tensor_mul(t1[:, 0:hw], raw[:, 0:hw],
                                         cosq_sb[:, h0:h0 + hw])
                    nc.vector.tensor_mul(t2[:, 0:hw], swp[:, 0:hw],
                                         sinq_sb[:, h0:h0 + hw])
                    nc.vector.tensor_add(qt_sb[m][:, h0:h0 + hw],
                                         t1[:, 0:hw], t2[:, 0:hw])

        # wo tiles: loaded during attention (HBM is idle then)
        es_wo = ExitStack()
        wop = es_wo.enter_context(tc.tile_pool(name="wo", bufs=1))
        wo_sb = []
        for f in range(2 * HL):
            t_ = wop.tile([P, cE // 2], BF, tag=f"wo{f}", name=f"wo{f}")
            nc.sync.dma_start(t_[:], wo_d[f * P:(f + 1) * P, :])
            wo_sb.append(t_)

        # ================= phase C: attention ============================
        FR = mybir.dt.float32r
        RPM = cTQ // P                # packed den rows per head
        # emission order of key chunks: wide uniform pairs, then leftover
        # uniform, then the NB biased trailing chunks
        wide_pairs = [(2 * i, 2 * i + 1) for i in range(NU // 2)]
        narrow = ([NU - 1] if NU % 2 else []) + list(range(NU, TKC))
        first_c = wide_pairs[0][0] if wide_pairs else narrow[0]
        last_c = narrow[-1] if narrow else wide_pairs[-1][1]

        with tc.tile_pool(name="ytp", bufs=2) as ytp, \
                tc.tile_pool(name="pt", bufs=4) as ptp, \
                tc.tile_pool(name="ptn", bufs=4) as ptnp, \
                tc.tile_pool(name="pt2", bufs=8) as pt2p, \
                tc.tile_pool(name="dst", bufs=2) as dstp, \
                tc.tile_pool(name="dner", bufs=2) as dnerp, \
                tc.tile_pool(name="pssw", bufs=2, space="PSUM") as pssw, \
                tc.tile_pool(name="psy", bufs=2, space="PSUM") as psy, \
                tc.tile_pool(name="psd", bufs=1, space="PSUM") as psd, \
                tc.tile_pool(name="psb", bufs=1, space="PSUM") as psb:
            for m in range(HL):
                yt = ytp.tile([P, cTQ], BF, tag="yt", name=f"yt{m}")
                for qs, qw in _cs(cTQ, NQ):
                    yps = psy.tile([P, NQ], F32, tag="yps", name="yps")
                    dps = psd.tile([1, NQ], F32, tag="dps", name="dps")
                    den_ops = []
                    eng_i = 0
                    for c0, c1 in wide_pairs:
                        sps = pssw.tile([P, 2 * NQ], F32, tag="sps",
                                        name="sps")
                        nc.tensor.matmul(
                            sps[:, 0:qw],
                            kt_sb[m][:, c0 * P:(c0 + 1) * P],
                            qt_sb[m][:, qs:qs + qw],
                            start=True, stop=True,
                        )
                        nc.tensor.matmul(
                            sps[:, NQ:NQ + qw],
                            kt_sb[m][:, c1 * P:(c1 + 1) * P],
                            qt_sb[m][:, qs:qs + qw],
                            start=True, stop=True,
                        )
                        pt = ptp.tile([P, 2 * NQ], BF, tag="pt", name="pt")
                        nc.scalar.activation(
                            pt[:], sps[:],
                            mybir.ActivationFunctionType.Exp,
                            bias=0.0, scale=SCALE,
                        )
                        nc.tensor.matmul(
                            yps[:, 0:qw],
                            v_sb[c0][:, m * P:(m + 1) * P],
                            pt[:, 0:qw],
                            start=(c0 == first_c), stop=False,
                        )
                        nc.tensor.matmul(
                            yps[:, 0:qw],
                            v_sb[c1][:, m * P:(m + 1) * P],
                            pt[:, NQ:NQ + qw],
                            start=False, stop=(c1 == last_c),
                        )
                        pt2 = pt2p.tile([P, NQ], BF, tag="pt2", name="pt2")
                        eng = nc.vector if (eng_i % 2 == 0) else nc.gpsimd
                        eng_i += 1
                        eng.tensor_add(pt2[:, 0:qw], pt[:, 0:qw],
                                       pt[:, NQ:NQ + qw])
                        den_ops.append(pt2)
                    for cn in narrow:
                        sps = pssw.tile([P, 2 * NQ], F32, tag="sps",
                                        name="sps")
                        nc.tensor.matmul(
                            sps[:, 0:qw],
                            kt_sb[m][:, cn * P:(cn + 1) * P],
                            qt_sb[m][:, qs:qs + qw],
                            start=True, stop=True,
                        )
                        ptn = ptnp.tile([P, NQ], BF, tag="ptn", name="ptn")
                        bias = (mb_sb[:, cn - NU:cn - NU + 1]
                                if cn >= NU else 0.0)
                        nc.scalar.activation(
                            ptn[:, 0:qw], sps[:, 0:qw],
                            mybir.ActivationFunctionType.Exp,
                            bias=bias, scale=SCALE,
                        )
                        nc.tensor.matmul(
                            yps[:, 0:qw],
                            v_sb[cn][:, m * P:(m + 1) * P],
                            ptn[:, 0:qw],
                            start=(cn == first_c), stop=(cn == last_c),
                        )
                        den_ops.append(ptn)
                    # quad-reduce the denominator operands to <= 5, then
                    # ones-column matmuls accumulate into dps
                    while len(den_ops) > 5:
                        nxt = []
                        for i in range(0, len(den_ops) - 1, 2):
                            pt2 = pt2p.tile([P, NQ], BF, tag="pt2",
                                            name="pt2")
                            eng = nc.vector if (eng_i % 2 == 0) else nc.gpsimd
                            eng_i += 1
                            eng.tensor_add(pt2[:, 0:qw],
                                           den_ops[i][:, 0:qw],
                                           den_ops[i + 1][:, 0:qw])
                            nxt.append(pt2)
                        if len(den_ops) % 2:
                            nxt.append(den_ops[-1])
                        den_ops = nxt
                    for i, dop in enumerate(den_ops):
                        nc.tensor.matmul(
                            dps[0:1, 0:qw],
                            ones_bf[:, 0:1],
                            dop[:, 0:qw],
                            start=(i == 0), stop=(i == len(den_ops) - 1),
                        )
                    nc.vector.tensor_copy(yt[:, qs:qs + qw], yps[:, 0:qw])
                    dst = dstp.tile([1, NQ], F32, tag="dst", name="dst")
                    nc.vector.tensor_copy(dst[0:1, 0:qw], dps[0:1, 0:qw])
                    # scatter the denominator row into the packed layout
                    bp = (m % 4) * 32 + qs // P
                    c0_ = (m // 4) * P
                    nc.sync.dma_start(
                        den_sb[bp:bp + qw // P, c0_:c0_ + P], dst[0:1, 0:qw])
                # ---- head m normalization (overlaps head m+1 attention) ----
                bp = (m % 4) * 32
                c0_ = (m // 4) * P
                nc.vector.reciprocal(den_sb[bp:bp + RPM, c0_:c0_ + P],
                                     den_sb[bp:bp + RPM, c0_:c0_ + P])
                dner = dnerp.tile([1, cTQ], F32, tag="dner", name="dner")
                nc.sync.dma_start(dner[0:1, :],
                                  den_sb[bp:bp + RPM, c0_:c0_ + P])
                for qs, qw in _cs(cTQ, NQ):
                    dbc = psb.tile([P, NQ], F32, tag="dbc", name="dbc")
                    nc.tensor.matmul(
                        dbc[:, 0:qw],
                        ones_fr[0:1, :].bitcast(FR),
                        dner[0:1, qs:qs + qw].bitcast(FR),
                        start=True, stop=True,
                    )
                    nc.vector.tensor_mul(
                        yt[:, qs:qs + qw],
                        yt[:, qs:qs + qw],
                        dbc[:, 0:qw],
                    )
                # ship head m's output to the pair partner while later heads
                # are still computing
                nc.sync.dma_start(ytd[m][:], yt[:])
                nc.gpsimd.collective_compute(
                    "AllGather",
                    mybir.AluOpType.bypass,
                    replica_groups=groups,
                    ins=[ytd[m][:]],
                    outs=[ytg[m][:]],
                )

        # ================= phase D: out-projection =======================
        # full contraction over all 16 gathered heads; output = this core's
        # E-half. f-tile order (m asc, half) puts the last-finished head's
        # tiles at the end of each accumulation chain.
        NT = (cE // 2) // P
        with tc.tile_pool(name="yg", bufs=2) as ygp, \
                tc.tile_pool(name="oev", bufs=4) as oevp, \
                tc.tile_pool(name="pso", bufs=2, space="PSUM") as pso:
            for ms, mw in _cs(cTQ, 512):
                yg_sb = []
                for m in range(HL):
                    for hf in range(2):
                        t_ = ygp.tile([P, 512], BF, tag=f"yg{m}_{hf}",
                                      name=f"yg{m}_{hf}")
                        nc.sync.dma_start(
                            t_[:, 0:mw],
                            ytg[m][hf * P:(hf + 1) * P, ms:ms + mw])
                        yg_sb.append(t_)
                for n in range(NT):
                    ops = pso.tile([P, 512], F32, tag="ops", name="ops")
                    for f in range(2 * HL):
                        nc.tensor.matmul(
                            ops[:, 0:mw],
                            wo_sb[f][:, n * P:(n + 1) * P],
                            yg_sb[f][:, 0:mw],
                            start=(f == 0), stop=(f == 2 * HL - 1),
                        )
                    oev = oevp.tile([P, 512], BF, tag="oev", name="oev")
                    nc.scalar.copy(oev[:, 0:mw], ops[:, 0:mw])
                    nc.sync.dma_start(
                        out_d[n * P:(n + 1) * P, ms:ms + mw],
                        oev[:, 0:mw])
        es_wo.close()

    return nc


# ---------------------------------------------------------------------------
# host side
# ---------------------------------------------------------------------------

def _rope_tables():
    inv_freq = 1.0 / (THETA ** (np.arange(0, D, 2, dtype=np.float32) / D))
    t = np.arange(BLOCK, dtype=np.float32)
    freqs = np.einsum("i,j->ij", t, inv_freq).astype(np.float32)
    emb = np.concatenate([freqs, freqs], axis=-1)
    return np.cos(emb).astype(np.float32), np.sin(emb).astype(np.float32)


_NC_CACHE = {}


def _get_compiled(cfg_key=None):
    if cfg_key is None:
        cfg_key = _NC_CACHE.get("last_cfg", (FULL_CFG["TKC"], FULL_CFG["NB"]))
    if cfg_key not in _NC_CACHE:
        nc = build_nc({"TKC": cfg_key[0], "NB": cfg_key[1]})
        nc.compile()
        _NC_CACHE[cfg_key] = nc
    return _NC_CACHE[cfg_key]


def _bf(a):
    return np.ascontiguousarray(a).astype(BF16NP)


def prepare_in_maps(x, xall, posx, posxall, mask, Wq, Wk, Wv, Wo):
    x = np.asarray(x, dtype=np.float32)
    xall = np.asarray(xall, dtype=np.float32)
    posx = np.asarray(posx)
    posxall = np.asarray(posxall)
    mask = np.asarray(mask).astype(bool)
    Wq = np.asarray(Wq, dtype=np.float32)
    Wk = np.asarray(Wk, dtype=np.float32)
    Wv = np.asarray(Wv, dtype=np.float32)
    Wo = np.asarray(Wo, dtype=np.float32)

    cos_t, sin_t = _rope_tables()
    sign = np.ones((1, D), np.float32)
    sign[0, : D // 2] = -1.0

    F = (H * D) // 2  # 1024: per-core head-shard width

    # sort keys: unmasked first; drop fully-masked tail chunks
    orders = [np.argsort(mask[b], kind="stable") for b in range(B)]
    kept = [int((~mask[b]).sum()) for b in range(B)]
    TKC = max(-(-k // 128) for k in kept)
    NB = max(1, TKC - min(kept) // 128)
    TKP = TKC * P
    _NC_CACHE["last_cfg"] = (TKC, NB)

    # wo rows in (head m asc, half) interleaved order to match the
    # per-head AllGather layout [head m ; head m+8]
    NUg = TKC - NB
    rowperm = np.concatenate(
        [np.arange(g * D, (g + 1) * D)
         for mh in range(H // 2) for g in (mh, mh + H // 2)])

    in_maps = []
    for cc in range(N_CORES):
        b, hg = cc // 2, cc % 2
        sl = slice(hg * F, (hg + 1) * F)
        kidx = orders[b][:TKP]
        pk = posxall[b][kidx]
        cosq = _bf(cos_t[posx[b]].T)                    # [128, TQ]
        sinq = _bf((sin_t[posx[b]] * sign).T)
        cosk = _bf(cos_t[pk].T)
        sink = _bf((sin_t[pk] * sign).T)
        mb = np.zeros((P, NB), np.float32)
        for j in range(NB):
            ch = NUg + j
            mb[:, j] = np.where(mask[b][kidx[ch * P:(ch + 1) * P]],
                                np.float32(-60.0), np.float32(0.0))
        in_maps.append({
            "xt": _bf(x[b].T),
            "xat": _bf(xall[b].T[:, kidx]),
            "wq": _bf(Wq[:, sl]),
            "wk": _bf(Wk[:, sl]),
            "wv": _bf(Wv[:, sl]),
            "wo": _bf(Wo[rowperm][:, hg * (E // 2):(hg + 1) * (E // 2)]),
            "cosq": cosq, "sinq": sinq, "cosk": cosk, "sink": sink,
            "mbias": mb,
        })
    return in_maps


def assemble_out(results):
    out = np.empty((B, TQ, E), np.float32)
    outT = np.empty((E, TQ), np.float32)
    for b in range(B):
        for hg in range(2):
            outT[hg * (E // 2):(hg + 1) * (E // 2)] = \
                results[2 * b + hg]["out"].astype(np.float32)
        out[b] = outT.T
    return out


def kernel(x, xall, posx, posxall, mask, Wq, Wk, Wv, Wo):
    from concourse.bass_utils import run_bass_kernel_spmd

    in_maps = prepare_in_maps(x, xall, posx, posxall, mask, Wq, Wk, Wv, Wo)
    nc = _get_compiled(_NC_CACHE["last_cfg"])
    res = run_bass_kernel_spmd(nc, in_maps, list(range(N_CORES)), trace=False)
    return assemble_out(res.results)
